# revision 6
# baseline (speedup 1.0000x reference)
"""CFNO forward kernel for Trainium2 (8 NeuronCores, data-parallel over batch).

The reference computes, per 16x16 patch p (flattened to 256):
    fft = FFT_256(p) (ortho); fc = fft @ Wc^T + bc; y = Re(IFFT_16(fc)) (ortho)
    z = y @ conv_w^T + conv_b;  out = GroupNorm_8(z) * gamma + beta

Because p is real and every step before GroupNorm is linear, the whole chain
folds into one real matrix on the host:
    M2 = Re(F @ Wc^T @ G) @ conv_w^T   [256, 16]
    b2 = Re(bc @ G) @ conv_w^T + conv_b [16]
    z  = p @ M2 + b2
(F = symmetric 256-pt DFT matrix / sqrt(256); G = inverse 16-pt DFT / sqrt(16))

On-device per core (one batch image, x [2048, 2048]):
  - 8 row-tiles of 256 image rows; SBUF layout [128 part=(hblk, s1), 2, 2048]
  - per tile, 16 PSUM-accumulating matmuls (one per patch-column offset s2)
    with a block-diagonal lhsT so all 8 h-blocks share one matmul
  - fused bias add via ScalarE on PSUM->SBUF copy; bn_stats for moments
  - one mask-matmul does the grouped cross-partition reduce AND broadcast
  - fused (z * A + B) normalize via ScalarE, single strided DMA out per tile
"""

import numpy as np
from contextlib import ExitStack

CHUNK = 16
GROUPS = 8
EPS = 1e-5
B, C, H, W = 8, 1, 2048, 2048
D = 16
D_IN = CHUNK * CHUNK * C  # 256
HP = H // CHUNK  # 128 patch rows
WP = W // CHUNK  # 128 patch cols
P = 128
N_TILES = 8  # row tiles of 256 image rows per core
N_CORES = 8

_CACHED_NC = None


def _build_nc():
    import concourse.tile as tile
    from concourse import bacc, mybir

    f32 = mybir.dt.float32
    nc = bacc.Bacc("TRN2", target_bir_lowering=False, debug=False,
                   num_devices=N_CORES)

    x = nc.dram_tensor("x", [H, W], f32, kind="ExternalInput").ap()
    wl = nc.dram_tensor("wl", [CHUNK, P, P], f32, kind="ExternalInput").ap()
    gmask = nc.dram_tensor("gmask", [P, P], f32, kind="ExternalInput").ap()
    consts = nc.dram_tensor("consts", [P, 3], f32, kind="ExternalInput").ap()
    # [hi, e, w] row-major: hi = ti*16 + half*8 + hblk, so partition (hblk, e)
    # and free (ti, half, w) each collapse to <=3 balanced DMA dims.
    # Host transposes to [D, HP, WP] afterward.
    out = nc.dram_tensor("out", [HP, D, WP], f32, kind="ExternalOutput").ap()

    Ident = mybir.ActivationFunctionType.Identity
    Sqrt = mybir.ActivationFunctionType.Sqrt

    with tile.TileContext(nc) as tc, ExitStack() as ctx:
        const_pool = ctx.enter_context(tc.tile_pool(name="const", bufs=1))
        xin = ctx.enter_context(tc.tile_pool(name="xin", bufs=3))
        zpool = ctx.enter_context(tc.tile_pool(name="z", bufs=1))
        opool = ctx.enter_context(tc.tile_pool(name="o", bufs=3))
        psum = ctx.enter_context(tc.tile_pool(name="psum", bufs=4, space="PSUM"))
        psg = ctx.enter_context(tc.tile_pool(name="psg", bufs=1, space="PSUM"))

        wtile = const_pool.tile([P, CHUNK, P], f32)
        nc.sync.dma_start(out=wtile, in_=wl.rearrange("s p m -> p s m"))
        gmt = const_pool.tile([P, P], f32)
        nc.sync.dma_start(out=gmt, in_=gmask)
        cvt = const_pool.tile([P, 3], f32)
        nc.sync.dma_start(out=cvt, in_=consts)
        epst = const_pool.tile([P, 1], f32)
        nc.vector.memset(epst, EPS)

        zall = zpool.tile([P, N_TILES, 2 * WP], f32)
        statsall = zpool.tile([P, N_TILES, nc.vector.BN_STATS_DIM], f32)

        # image row = ti*256 + half*128 + hblk*16 + s1
        xr = x.rearrange("(t half hblk s1) c -> t (hblk s1) half c",
                         t=N_TILES, half=2, hblk=8, s1=CHUNK)
        for ti in range(N_TILES):
            xt = xin.tile([P, 2, W], f32, tag="xt")
            nc.sync.dma_start(out=xt, in_=xr[ti])
            xs = xt.rearrange("p h (w s) -> p h w s", s=CHUNK)
            pt = psum.tile([P, 2, WP], f32, tag="pt")
            for s2 in range(CHUNK):
                nc.tensor.matmul(pt, lhsT=wtile[:, s2, :], rhs=xs[:, :, :, s2],
                                 start=(s2 == 0), stop=(s2 == CHUNK - 1))
            # z (+bias) -> SBUF; partition (hblk, e), free (half, w)
            nc.scalar.activation(out=zall[:, ti],
                                 in_=pt.rearrange("p a b -> p (a b)"),
                                 func=Ident, bias=cvt[:, 0:1])
            nc.vector.bn_stats(out=statsall[:, ti], in_=zall[:, ti])

        # Per-partition mean/var over all 2048 elements
        mv = zpool.tile([P, 2], f32)
        nc.vector.bn_aggr(out=mv, in_=statsall)
        # rhs = (mean_p, E[x^2]_p)
        msq = zpool.tile([P, 1], f32)
        nc.vector.tensor_mul(msq, mv[:, 0:1], mv[:, 0:1])
        me2 = zpool.tile([P, 2], f32)
        nc.vector.tensor_copy(me2[:, 0:1], mv[:, 0:1])
        nc.vector.tensor_add(me2[:, 1:2], mv[:, 1:2], msq)
        # Grouped cross-partition average + broadcast in one matmul:
        # gp[p'] = (1/16) * sum_{p in group(p')} me2[p]
        gp = psg.tile([P, 2], f32)
        nc.tensor.matmul(gp, lhsT=gmt, rhs=me2, start=True, stop=True)
        gsb = zpool.tile([P, 2], f32)
        nc.vector.tensor_copy(gsb, gp)
        gmean = gsb[:, 0:1]
        gmsq = zpool.tile([P, 1], f32)
        nc.vector.tensor_mul(gmsq, gmean, gmean)
        gvar = zpool.tile([P, 1], f32)
        nc.vector.tensor_sub(gvar, gsb[:, 1:2], gmsq)
        sd = zpool.tile([P, 1], f32)
        nc.scalar.activation(out=sd, in_=gvar, func=Sqrt, bias=epst)
        rs = zpool.tile([P, 1], f32)
        nc.vector.reciprocal(rs, sd)
        # out = z * A + Bp with A = rsqrt*gamma, Bp = beta - mean*A
        A = zpool.tile([P, 1], f32)
        nc.vector.tensor_mul(A, rs, cvt[:, 1:2])
        mA = zpool.tile([P, 1], f32)
        nc.vector.tensor_mul(mA, gmean, A)
        Bp = zpool.tile([P, 1], f32)
        nc.vector.tensor_sub(Bp, cvt[:, 2:3], mA)

        orr = out.rearrange("(t half hblk) e w -> (hblk e) (t half) w",
                            t=N_TILES, half=2, hblk=8)
        for ti in range(N_TILES):
            ot = opool.tile([P, 2, WP], f32, tag="ot")
            nc.scalar.activation(out=ot.rearrange("p a b -> p (a b)"),
                                 in_=zall[:, ti], func=Ident,
                                 scale=A, bias=Bp)
            nc.sync.dma_start(out=orr[:, 2 * ti:2 * ti + 2, :], in_=ot)

    nc.compile()
    return nc


def _host_weights(fc_wr, fc_wi, fc_br, fc_bi, conv_w, conv_b, gamma, beta):
    fc_wr = np.asarray(fc_wr, np.float64)
    fc_wi = np.asarray(fc_wi, np.float64)
    fc_br = np.asarray(fc_br, np.float64)
    fc_bi = np.asarray(fc_bi, np.float64)
    conv_w = np.asarray(conv_w, np.float64)
    conv_b = np.asarray(conv_b, np.float64)
    gamma = np.asarray(gamma, np.float64)
    beta = np.asarray(beta, np.float64)

    j = np.arange(D_IN)
    F = np.exp(-2j * np.pi * np.outer(j, j) / D_IN) / np.sqrt(D_IN)
    d = np.arange(D)
    G = np.exp(2j * np.pi * np.outer(d, d) / D) / np.sqrt(D)
    Wc = fc_wr + 1j * fc_wi
    bc = fc_br + 1j * fc_bi
    M2 = (np.real(F @ Wc.T @ G) @ conv_w.T).astype(np.float32)  # [256, 16]
    b2 = (np.real(bc @ G) @ conv_w.T + conv_b).astype(np.float32)  # [16]

    # Block-diagonal lhsT: wl[s2, hblk*16+s1, hblk*16+e] = M2[s1*16+s2, e]
    wl = np.zeros((CHUNK, P, P), np.float32)
    blk = M2.reshape(CHUNK, CHUNK, D).transpose(1, 0, 2)  # [s2, s1, e]
    for hb in range(8):
        wl[:, hb * 16:hb * 16 + 16, hb * 16:hb * 16 + 16] = blk

    # Group-average + broadcast mask (1/16 where groups match)
    pidx = np.arange(P)
    grp = (pidx % D) // (D // GROUPS)
    gmask = (grp[:, None] == grp[None, :]).astype(np.float32) / 16.0

    e = pidx % D
    consts = np.stack([b2[e], gamma.astype(np.float32)[e],
                       beta.astype(np.float32)[e]], axis=1)  # [128, 3]
    return wl, gmask, consts


def kernel(x, fc_wr, fc_wi, fc_br, fc_bi, conv_w, conv_b, gamma, beta,
           _return_results=False, _trace=False):
    from concourse.bass_utils import run_bass_kernel_spmd

    global _CACHED_NC
    if _CACHED_NC is None:
        _CACHED_NC = _build_nc()
    nc = _CACHED_NC

    wl, gmask, consts = _host_weights(fc_wr, fc_wi, fc_br, fc_bi,
                                      conv_w, conv_b, gamma, beta)
    x = np.ascontiguousarray(np.asarray(x, np.float32).reshape(B, H, W))
    in_maps = [{"x": x[b], "wl": wl, "gmask": gmask, "consts": consts}
               for b in range(N_CORES)]
    res = run_bass_kernel_spmd(nc, in_maps, list(range(N_CORES)),
                               trace=_trace)
    # device layout is [HP, D, WP]; grade layout is [B, D, HP, WP]
    out = np.stack([res.results[b]["out"].transpose(1, 0, 2)
                    for b in range(N_CORES)], axis=0)
    if _return_results:
        return out, res
    return out


# revision 13
# speedup vs baseline: 1.7979x; 1.7979x over previous
"""CFNO forward kernel for Trainium2 (8 NeuronCores, data-parallel over batch).

The reference computes, per 16x16 patch p (flattened to 256):
    fft = FFT_256(p) (ortho); fc = fft @ Wc^T + bc; y = Re(IFFT_16(fc)) (ortho)
    z = y @ conv_w^T + conv_b;  out = GroupNorm_8(z) * gamma + beta

Because p is real and every step before GroupNorm is linear, the whole chain
folds into one real matrix on the host:
    M2 = Re(F @ Wc^T @ G) @ conv_w^T   [256, 16]
    b2 = Re(bc @ G) @ conv_w^T + conv_b [16]
    z  = p @ M2 + b2
(F = symmetric 256-pt DFT matrix / sqrt(256); G = inverse 16-pt DFT / sqrt(16))

On-device per core (one batch image, x [2048, 2048]):
  - 8 row-tiles of 256 image rows; SBUF layout [128 part=(hblk, s1), 2, 2048]
  - per tile, 16 PSUM-accumulating matmuls (one per patch-column offset s2)
    with a block-diagonal lhsT so all 8 h-blocks share one matmul
  - fused bias add via ScalarE on PSUM->SBUF copy; bn_stats for moments
  - one mask-matmul does the grouped cross-partition reduce AND broadcast
  - fused (z * A + B) normalize via ScalarE, single strided DMA out per tile
"""

import numpy as np
from contextlib import ExitStack

CHUNK = 16
GROUPS = 8
EPS = 1e-5
B, C, H, W = 8, 1, 2048, 2048
D = 16
D_IN = CHUNK * CHUNK * C  # 256
HP = H // CHUNK  # 128 patch rows
WP = W // CHUNK  # 128 patch cols
P = 128
N_TILES = 8  # row tiles of 256 image rows per core
N_CORES = 8

_CACHED_NC = {}


def _build_nc(mm_dtype="float32r"):
    import concourse.tile as tile
    from concourse import bacc, mybir

    f32 = mybir.dt.float32
    mmdt = getattr(mybir.dt, mm_dtype)
    nc = bacc.Bacc("TRN2", target_bir_lowering=False, debug=False,
                   num_devices=N_CORES)

    x = nc.dram_tensor("x", [H, W], mmdt, kind="ExternalInput").ap()
    wl = nc.dram_tensor("wl", [CHUNK, P, P], mmdt, kind="ExternalInput").ap()
    gmask = nc.dram_tensor("gmask", [P, P], f32, kind="ExternalInput").ap()
    consts = nc.dram_tensor("consts", [P, 3], f32, kind="ExternalInput").ap()
    # [hi, e, w] row-major: hi = ti*16 + half*8 + hblk, so partition (hblk, e)
    # and free (ti, half, w) each collapse to <=3 balanced DMA dims.
    # Host transposes to [D, HP, WP] afterward.
    out = nc.dram_tensor("out", [HP, D, WP], f32, kind="ExternalOutput").ap()

    Ident = mybir.ActivationFunctionType.Identity
    Sqrt = mybir.ActivationFunctionType.Sqrt

    with tile.TileContext(nc) as tc, ExitStack() as ctx:
        const_pool = ctx.enter_context(tc.tile_pool(name="const", bufs=1))
        xin = ctx.enter_context(tc.tile_pool(name="xin", bufs=3))
        zpool = ctx.enter_context(tc.tile_pool(name="z", bufs=1))
        opool = ctx.enter_context(tc.tile_pool(name="o", bufs=3))
        psum = ctx.enter_context(tc.tile_pool(name="psum", bufs=4, space="PSUM"))
        psg = ctx.enter_context(tc.tile_pool(name="psg", bufs=1, space="PSUM"))

        wtile = const_pool.tile([P, CHUNK, P], mmdt)
        nc.sync.dma_start(out=wtile, in_=wl.rearrange("s p m -> p s m"))
        gmt = const_pool.tile([P, P], f32)
        nc.sync.dma_start(out=gmt, in_=gmask)
        cvt = const_pool.tile([P, 3], f32)
        nc.sync.dma_start(out=cvt, in_=consts)
        epst = const_pool.tile([P, 1], f32)
        nc.vector.memset(epst, EPS)

        zall = zpool.tile([P, N_TILES, 2 * WP], f32)
        statsall = zpool.tile([P, N_TILES, nc.vector.BN_STATS_DIM], f32)

        # image row = ti*256 + half*128 + hblk*16 + s1
        xr = x.rearrange("(t half hblk s1) c -> t (hblk s1) half c",
                         t=N_TILES, half=2, hblk=8, s1=CHUNK)
        for ti in range(N_TILES):
            xt = xin.tile([P, 2, W], mmdt, tag="xt")
            nc.sync.dma_start(out=xt, in_=xr[ti])
            xs = xt.rearrange("p h (w s) -> p h w s", s=CHUNK)
            pt = psum.tile([P, 2, WP], f32, tag="pt")
            for s2 in range(CHUNK):
                nc.tensor.matmul(pt, lhsT=wtile[:, s2, :],
                                 rhs=xs[:, :, :, s2],
                                 start=(s2 == 0), stop=(s2 == CHUNK - 1))
            # z (+bias) -> SBUF; partition (hblk, e), free (half, w)
            nc.scalar.activation(out=zall[:, ti],
                                 in_=pt.rearrange("p a b -> p (a b)"),
                                 func=Ident, bias=cvt[:, 0:1])
            nc.vector.bn_stats(out=statsall[:, ti], in_=zall[:, ti])

        # Per-partition mean/var over all 2048 elements
        mv = zpool.tile([P, 2], f32)
        nc.vector.bn_aggr(out=mv, in_=statsall)
        # rhs = (mean_p, E[x^2]_p)
        msq = zpool.tile([P, 1], f32)
        nc.vector.tensor_mul(msq, mv[:, 0:1], mv[:, 0:1])
        me2 = zpool.tile([P, 2], f32)
        nc.vector.tensor_copy(me2[:, 0:1], mv[:, 0:1])
        nc.vector.tensor_add(me2[:, 1:2], mv[:, 1:2], msq)
        # Grouped cross-partition average + broadcast in one matmul:
        # gp[p'] = (1/16) * sum_{p in group(p')} me2[p]
        gp = psg.tile([P, 2], f32)
        nc.tensor.matmul(gp, lhsT=gmt, rhs=me2, start=True, stop=True)
        gsb = zpool.tile([P, 2], f32)
        nc.vector.tensor_copy(gsb, gp)
        gmean = gsb[:, 0:1]
        gmsq = zpool.tile([P, 1], f32)
        nc.vector.tensor_mul(gmsq, gmean, gmean)
        gvar = zpool.tile([P, 1], f32)
        nc.vector.tensor_sub(gvar, gsb[:, 1:2], gmsq)
        sd = zpool.tile([P, 1], f32)
        nc.scalar.activation(out=sd, in_=gvar, func=Sqrt, bias=epst)
        rs = zpool.tile([P, 1], f32)
        nc.vector.reciprocal(rs, sd)
        # out = z * A + Bp with A = rsqrt*gamma, Bp = beta - mean*A
        A = zpool.tile([P, 1], f32)
        nc.vector.tensor_mul(A, rs, cvt[:, 1:2])
        mA = zpool.tile([P, 1], f32)
        nc.vector.tensor_mul(mA, gmean, A)
        Bp = zpool.tile([P, 1], f32)
        nc.vector.tensor_sub(Bp, cvt[:, 2:3], mA)

        orr = out.rearrange("(t half hblk) e w -> (hblk e) (t half) w",
                            t=N_TILES, half=2, hblk=8)
        for ti in range(N_TILES):
            ot = opool.tile([P, 2, WP], f32, tag="ot")
            nc.scalar.activation(out=ot.rearrange("p a b -> p (a b)"),
                                 in_=zall[:, ti], func=Ident,
                                 scale=A, bias=Bp)
            nc.sync.dma_start(out=orr[:, 2 * ti:2 * ti + 2, :], in_=ot)

    nc.compile()
    return nc


def _host_weights(fc_wr, fc_wi, fc_br, fc_bi, conv_w, conv_b, gamma, beta):
    fc_wr = np.asarray(fc_wr, np.float64)
    fc_wi = np.asarray(fc_wi, np.float64)
    fc_br = np.asarray(fc_br, np.float64)
    fc_bi = np.asarray(fc_bi, np.float64)
    conv_w = np.asarray(conv_w, np.float64)
    conv_b = np.asarray(conv_b, np.float64)
    gamma = np.asarray(gamma, np.float64)
    beta = np.asarray(beta, np.float64)

    j = np.arange(D_IN)
    F = np.exp(-2j * np.pi * np.outer(j, j) / D_IN) / np.sqrt(D_IN)
    d = np.arange(D)
    G = np.exp(2j * np.pi * np.outer(d, d) / D) / np.sqrt(D)
    Wc = fc_wr + 1j * fc_wi
    bc = fc_br + 1j * fc_bi
    M2 = (np.real(F @ Wc.T @ G) @ conv_w.T).astype(np.float32)  # [256, 16]
    b2 = (np.real(bc @ G) @ conv_w.T + conv_b).astype(np.float32)  # [16]

    # Block-diagonal lhsT: wl[s2, hblk*16+s1, hblk*16+e] = M2[s1*16+s2, e]
    wl = np.zeros((CHUNK, P, P), np.float32)
    blk = M2.reshape(CHUNK, CHUNK, D).transpose(1, 0, 2)  # [s2, s1, e]
    for hb in range(8):
        wl[:, hb * 16:hb * 16 + 16, hb * 16:hb * 16 + 16] = blk

    # Group-average + broadcast mask (1/16 where groups match)
    pidx = np.arange(P)
    grp = (pidx % D) // (D // GROUPS)
    gmask = (grp[:, None] == grp[None, :]).astype(np.float32) / 16.0

    e = pidx % D
    consts = np.stack([b2[e], gamma.astype(np.float32)[e],
                       beta.astype(np.float32)[e]], axis=1)  # [128, 3]
    return wl, gmask, consts


def kernel(x, fc_wr, fc_wi, fc_br, fc_bi, conv_w, conv_b, gamma, beta,
           _return_results=False, _trace=False, _mm_dtype="float32r"):
    from concourse.bass_utils import run_bass_kernel_spmd

    if _mm_dtype not in _CACHED_NC:
        _CACHED_NC[_mm_dtype] = _build_nc(_mm_dtype)
    nc = _CACHED_NC[_mm_dtype]

    wl, gmask, consts = _host_weights(fc_wr, fc_wi, fc_br, fc_bi,
                                      conv_w, conv_b, gamma, beta)
    x = np.ascontiguousarray(np.asarray(x, np.float32).reshape(B, H, W))
    in_maps = [{"x": x[b], "wl": wl, "gmask": gmask, "consts": consts}
               for b in range(N_CORES)]
    res = run_bass_kernel_spmd(nc, in_maps, list(range(N_CORES)),
                               trace=_trace)
    # device layout is [HP, D, WP]; grade layout is [B, D, HP, WP]
    out = np.stack([res.results[b]["out"].transpose(1, 0, 2)
                    for b in range(N_CORES)], axis=0)
    if _return_results:
        return out, res
    return out


# revision 14
# speedup vs baseline: 1.9945x; 1.1093x over previous
"""CFNO forward kernel for Trainium2 (8 NeuronCores, data-parallel over batch).

The reference computes, per 16x16 patch p (flattened to 256):
    fft = FFT_256(p) (ortho); fc = fft @ Wc^T + bc; y = Re(IFFT_16(fc)) (ortho)
    z = y @ conv_w^T + conv_b;  out = GroupNorm_8(z) * gamma + beta

Because p is real and every step before GroupNorm is linear, the whole chain
folds into one real matrix on the host:
    M2 = Re(F @ Wc^T @ G) @ conv_w^T   [256, 16]
    b2 = Re(bc @ G) @ conv_w^T + conv_b [16]
    z  = p @ M2 + b2
(F = symmetric 256-pt DFT matrix / sqrt(256); G = inverse 16-pt DFT / sqrt(16))

On-device per core (one batch image, x [2048, 2048]):
  - 4 row-tiles of 512 image rows; SBUF layout [128 part=(hblk, s1), 4, 2048]
  - per tile, 16 PSUM-accumulating matmuls (one per patch-column offset s2,
    free dim 512) with a block-diagonal lhsT so all 8 h-blocks share a matmul;
    float32r keeps full fp32 storage with a fast (TF32-like) PE mode
  - fused bias add via ScalarE on PSUM->SBUF copy; bn_stats for moments
  - one mask-matmul does the grouped cross-partition reduce AND broadcast
  - fused (z * A + B) normalize via ScalarE, one contiguous 1 MB DMA out
"""

import numpy as np
from contextlib import ExitStack

CHUNK = 16
GROUPS = 8
EPS = 1e-5
B, C, H, W = 8, 1, 2048, 2048
D = 16
D_IN = CHUNK * CHUNK * C  # 256
HP = H // CHUNK  # 128 patch rows
WP = W // CHUNK  # 128 patch cols
P = 128
N_TILES = 4   # row tiles of 512 image rows per core
QH = 4        # 128-row quarter blocks per tile
N_CORES = 8

_CACHED_NC = {}


def _build_nc(mm_dtype="float32r"):
    import concourse.tile as tile
    from concourse import bacc, mybir

    f32 = mybir.dt.float32
    mmdt = getattr(mybir.dt, mm_dtype)
    nc = bacc.Bacc("TRN2", target_bir_lowering=False, debug=False,
                   num_devices=N_CORES)

    x = nc.dram_tensor("x", [H, W], mmdt, kind="ExternalInput").ap()
    # host-packed [p, s2, m] so the SBUF load is contiguous per partition
    wl = nc.dram_tensor("wl", [P, CHUNK * P], mmdt, kind="ExternalInput").ap()
    gmask = nc.dram_tensor("gmask", [P, P], f32, kind="ExternalInput").ap()
    consts = nc.dram_tensor("consts", [P, 3], f32, kind="ExternalInput").ap()
    # [p=(hblk,e), ti, q, w] flattened; host reorders to [D, HP, WP]
    out = nc.dram_tensor("out", [P, N_TILES * QH * WP], f32,
                         kind="ExternalOutput").ap()

    Ident = mybir.ActivationFunctionType.Identity
    Sqrt = mybir.ActivationFunctionType.Sqrt
    assert nc.vector.BN_STATS_FMAX >= QH * WP

    with tile.TileContext(nc) as tc, ExitStack() as ctx:
        const_pool = ctx.enter_context(tc.tile_pool(name="const", bufs=1))
        xin = ctx.enter_context(tc.tile_pool(name="xin", bufs=3))
        zpool = ctx.enter_context(tc.tile_pool(name="z", bufs=1))
        psum = ctx.enter_context(tc.tile_pool(name="psum", bufs=4, space="PSUM"))
        psg = ctx.enter_context(tc.tile_pool(name="psg", bufs=1, space="PSUM"))

        wtile = const_pool.tile([P, CHUNK, P], mmdt)
        nc.sync.dma_start(out=wtile.rearrange("p s m -> p (s m)"), in_=wl)
        gmt = const_pool.tile([P, P], f32)
        nc.sync.dma_start(out=gmt, in_=gmask)
        cvt = const_pool.tile([P, 3], f32)
        nc.sync.dma_start(out=cvt, in_=consts)
        epst = const_pool.tile([P, 1], f32)
        nc.vector.memset(epst, EPS)

        zall = zpool.tile([P, N_TILES, QH * WP], f32)
        statsall = zpool.tile([P, N_TILES, nc.vector.BN_STATS_DIM], f32)

        # image row = ti*512 + q*128 + hblk*16 + s1
        xr = x.rearrange("(t q hblk s1) c -> t (hblk s1) q c",
                         t=N_TILES, q=QH, hblk=8, s1=CHUNK)
        for ti in range(N_TILES):
            xt = xin.tile([P, QH, W], mmdt, tag="xt")
            nc.sync.dma_start(out=xt, in_=xr[ti])
            xs = xt.rearrange("p q (w s) -> p q w s", s=CHUNK)
            pt = psum.tile([P, QH, WP], f32, tag="pt")
            for s2 in range(CHUNK):
                nc.tensor.matmul(pt, lhsT=wtile[:, s2, :],
                                 rhs=xs[:, :, :, s2],
                                 start=(s2 == 0), stop=(s2 == CHUNK - 1))
            # z (+bias) -> SBUF; partition (hblk, e), free (q, w)
            nc.scalar.activation(out=zall[:, ti],
                                 in_=pt.rearrange("p a b -> p (a b)"),
                                 func=Ident, bias=cvt[:, 0:1])
            nc.vector.bn_stats(out=statsall[:, ti], in_=zall[:, ti])

        # Per-partition mean/var over all 2048 elements
        mv = zpool.tile([P, 2], f32)
        nc.vector.bn_aggr(out=mv, in_=statsall)
        # rhs = (mean_p, E[x^2]_p)
        msq = zpool.tile([P, 1], f32)
        nc.vector.tensor_mul(msq, mv[:, 0:1], mv[:, 0:1])
        me2 = zpool.tile([P, 2], f32)
        nc.vector.tensor_copy(me2[:, 0:1], mv[:, 0:1])
        nc.vector.tensor_add(me2[:, 1:2], mv[:, 1:2], msq)
        # Grouped cross-partition average + broadcast in one matmul:
        # gp[p'] = (1/16) * sum_{p in group(p')} me2[p]
        gp = psg.tile([P, 2], f32)
        nc.tensor.matmul(gp, lhsT=gmt, rhs=me2, start=True, stop=True)
        gsb = zpool.tile([P, 2], f32)
        nc.vector.tensor_copy(gsb, gp)
        gmean = gsb[:, 0:1]
        gmsq = zpool.tile([P, 1], f32)
        nc.vector.tensor_mul(gmsq, gmean, gmean)
        gvar = zpool.tile([P, 1], f32)
        nc.vector.tensor_sub(gvar, gsb[:, 1:2], gmsq)
        sd = zpool.tile([P, 1], f32)
        nc.scalar.activation(out=sd, in_=gvar, func=Sqrt, bias=epst)
        rs = zpool.tile([P, 1], f32)
        nc.vector.reciprocal(rs, sd)
        # out = z * A + Bp with A = rsqrt*gamma, Bp = beta - mean*A
        A = zpool.tile([P, 1], f32)
        nc.vector.tensor_mul(A, rs, cvt[:, 1:2])
        mA = zpool.tile([P, 1], f32)
        nc.vector.tensor_mul(mA, gmean, A)
        Bp = zpool.tile([P, 1], f32)
        nc.vector.tensor_sub(Bp, cvt[:, 2:3], mA)

        onorm = zpool.tile([P, N_TILES, QH * WP], f32)
        for ti in range(N_TILES):
            nc.scalar.activation(out=onorm[:, ti], in_=zall[:, ti],
                                 func=Ident, scale=A, bias=Bp)
        nc.sync.dma_start(out=out,
                          in_=onorm.rearrange("p a b -> p (a b)"))

    nc.compile()
    return nc


def _host_weights(fc_wr, fc_wi, fc_br, fc_bi, conv_w, conv_b, gamma, beta):
    fc_wr = np.asarray(fc_wr, np.float64)
    fc_wi = np.asarray(fc_wi, np.float64)
    fc_br = np.asarray(fc_br, np.float64)
    fc_bi = np.asarray(fc_bi, np.float64)
    conv_w = np.asarray(conv_w, np.float64)
    conv_b = np.asarray(conv_b, np.float64)
    gamma = np.asarray(gamma, np.float64)
    beta = np.asarray(beta, np.float64)

    j = np.arange(D_IN)
    F = np.exp(-2j * np.pi * np.outer(j, j) / D_IN) / np.sqrt(D_IN)
    d = np.arange(D)
    G = np.exp(2j * np.pi * np.outer(d, d) / D) / np.sqrt(D)
    Wc = fc_wr + 1j * fc_wi
    bc = fc_br + 1j * fc_bi
    M2 = (np.real(F @ Wc.T @ G) @ conv_w.T).astype(np.float32)  # [256, 16]
    b2 = (np.real(bc @ G) @ conv_w.T + conv_b).astype(np.float32)  # [16]

    # Block-diagonal lhsT: wl[hblk*16+s1, s2, hblk*16+e] = M2[s1*16+s2, e],
    # packed [p, s2*128+m] for a contiguous per-partition SBUF load.
    wl = np.zeros((CHUNK, P, P), np.float32)  # [s2, p, m]
    blk = M2.reshape(CHUNK, CHUNK, D).transpose(1, 0, 2)  # [s2, s1, e]
    for hb in range(8):
        wl[:, hb * 16:hb * 16 + 16, hb * 16:hb * 16 + 16] = blk
    wl = np.ascontiguousarray(wl.transpose(1, 0, 2).reshape(P, CHUNK * P))

    # Group-average + broadcast mask (1/16 where groups match)
    pidx = np.arange(P)
    grp = (pidx % D) // (D // GROUPS)
    gmask = (grp[:, None] == grp[None, :]).astype(np.float32) / 16.0

    e = pidx % D
    consts = np.stack([b2[e], gamma.astype(np.float32)[e],
                       beta.astype(np.float32)[e]], axis=1)  # [128, 3]
    return wl, gmask, consts


def kernel(x, fc_wr, fc_wi, fc_br, fc_bi, conv_w, conv_b, gamma, beta,
           _return_results=False, _trace=False, _mm_dtype="float32r"):
    from concourse.bass_utils import run_bass_kernel_spmd

    if _mm_dtype not in _CACHED_NC:
        _CACHED_NC[_mm_dtype] = _build_nc(_mm_dtype)
    nc = _CACHED_NC[_mm_dtype]

    wl, gmask, consts = _host_weights(fc_wr, fc_wi, fc_br, fc_bi,
                                      conv_w, conv_b, gamma, beta)
    x = np.ascontiguousarray(np.asarray(x, np.float32).reshape(B, H, W))
    in_maps = [{"x": x[b], "wl": wl, "gmask": gmask, "consts": consts}
               for b in range(N_CORES)]
    res = run_bass_kernel_spmd(nc, in_maps, list(range(N_CORES)),
                               trace=_trace)
    # device layout [p=(hblk,e), ti, q, w] -> [D, HP, WP], hi = ti*32+q*8+hblk
    out = np.stack(
        [res.results[b]["out"].reshape(8, D, N_TILES, QH, WP)
         .transpose(1, 2, 3, 0, 4).reshape(D, HP, WP)
         for b in range(N_CORES)], axis=0)
    if _return_results:
        return out, res
    return out


# revision 19
# speedup vs baseline: 2.1294x; 1.0677x over previous
"""CFNO forward kernel for Trainium2 (8 NeuronCores, data-parallel over batch).

The reference computes, per 16x16 patch p (flattened to 256):
    fft = FFT_256(p) (ortho); fc = fft @ Wc^T + bc; y = Re(IFFT_16(fc)) (ortho)
    z = y @ conv_w^T + conv_b;  out = GroupNorm_8(z) * gamma + beta

Because p is real and every step before GroupNorm is linear, the whole chain
folds into one real matrix on the host:
    M2 = Re(F @ Wc^T @ G) @ conv_w^T   [256, 16]
    b2 = Re(bc @ G) @ conv_w^T + conv_b [16]
    z  = p @ M2 + b2
(F = symmetric 256-pt DFT matrix / sqrt(256); G = inverse 16-pt DFT / sqrt(16))

On-device per core (one batch image, x [2048, 2048]):
  - 4 row-tiles of 512 image rows; SBUF layout [128 part=(hblk, s1), 4, 2048]
  - per tile, 16 PSUM-accumulating matmuls (one per patch-column offset s2,
    free dim 512) with a block-diagonal lhsT so all 8 h-blocks share a matmul;
    float32r keeps full fp32 storage with a fast (TF32-like) PE mode
  - fused bias add via ScalarE on PSUM->SBUF copy; bn_stats for moments
  - one mask-matmul does the grouped cross-partition reduce AND broadcast
  - fused (z * A + B) normalize via ScalarE, one contiguous 1 MB DMA out
"""

import numpy as np
from contextlib import ExitStack

CHUNK = 16
GROUPS = 8
EPS = 1e-5
B, C, H, W = 8, 1, 2048, 2048
D = 16
D_IN = CHUNK * CHUNK * C  # 256
HP = H // CHUNK  # 128 patch rows
WP = W // CHUNK  # 128 patch cols
P = 128
# row tiles per core, in 128-image-row blocks; big tiles stream first, small
# tiles at the end shrink the compute tail after the last input byte lands
TILE_Q = [4, 4, 4, 2, 1, 1]
RB = 16  # total 128-row blocks (2048 rows)
N_CORES = 8

_CACHED_NC = {}


def _build_nc(mm_dtype="float32r"):
    import concourse.tile as tile
    from concourse import bacc, mybir

    f32 = mybir.dt.float32
    mmdt = getattr(mybir.dt, mm_dtype)
    nc = bacc.Bacc("TRN2", target_bir_lowering=False, debug=False,
                   num_devices=N_CORES)

    x = nc.dram_tensor("x", [H, W], mmdt, kind="ExternalInput").ap()
    # host-packed [p, s2, m] so the SBUF load is contiguous per partition
    wl = nc.dram_tensor("wl", [P, CHUNK * P], mmdt, kind="ExternalInput").ap()
    gmask = nc.dram_tensor("gmask", [P, P], f32, kind="ExternalInput").ap()
    consts = nc.dram_tensor("consts", [P, 3], f32, kind="ExternalInput").ap()
    # [p=(hblk,e), rg, w] flattened (rg = hi//8); host reorders to [D, HP, WP]
    out = nc.dram_tensor("out", [P, RB * WP], f32, kind="ExternalOutput").ap()

    Ident = mybir.ActivationFunctionType.Identity
    Sqrt = mybir.ActivationFunctionType.Sqrt
    n_tiles = len(TILE_Q)
    assert sum(TILE_Q) == RB
    assert nc.vector.BN_STATS_FMAX >= max(TILE_Q) * WP

    with tile.TileContext(nc) as tc, ExitStack() as ctx:
        const_pool = ctx.enter_context(tc.tile_pool(name="const", bufs=1))
        xin = ctx.enter_context(tc.tile_pool(name="xin", bufs=3))
        zpool = ctx.enter_context(tc.tile_pool(name="z", bufs=1))
        psum = ctx.enter_context(tc.tile_pool(name="psum", bufs=4, space="PSUM"))
        psg = ctx.enter_context(tc.tile_pool(name="psg", bufs=1, space="PSUM"))

        # x row-block rb covers image rows [rb*128, (rb+1)*128):
        # row = rb*128 + hblk*16 + s1
        xr = x.rearrange("(rb hblk s1) c -> rb (hblk s1) c",
                         rb=RB, hblk=8, s1=CHUNK)

        # first x tile leads the SP DMA ring; consts go on the ACT ring
        xts = []
        rg0s = []
        rg = 0
        for ti, q in enumerate(TILE_Q):
            rg0s.append(rg)
            xt = xin.tile([P, q, W], mmdt, tag="xt")
            nc.sync.dma_start(out=xt, in_=xr[rg:rg + q].transpose([1, 0, 2]))
            xts.append(xt)
            rg += q
            if ti == 0:
                wtile = const_pool.tile([P, CHUNK, P], mmdt)
                nc.scalar.dma_start(
                    out=wtile.rearrange("p s m -> p (s m)"), in_=wl)
                gmt = const_pool.tile([P, P], f32)
                nc.scalar.dma_start(out=gmt, in_=gmask)
                cvt = const_pool.tile([P, 3], f32)
                nc.scalar.dma_start(out=cvt, in_=consts)
                epst = const_pool.tile([P, 1], f32)
                nc.vector.memset(epst, EPS)

        zall = zpool.tile([P, RB * WP], f32)
        statsall = zpool.tile([P, n_tiles, nc.vector.BN_STATS_DIM], f32)

        for ti, q in enumerate(TILE_Q):
            xt = xts[ti]
            rg0 = rg0s[ti]
            xs = xt.rearrange("p q (w s) -> p q w s", s=CHUNK)
            pt_full = psum.tile([P, max(TILE_Q), WP], f32, tag="pt", name="pt")
            pt = pt_full[:, :q]
            for s2 in range(CHUNK):
                nc.tensor.matmul(pt, lhsT=wtile[:, s2, :],
                                 rhs=xs[:, :, :, s2],
                                 start=(s2 == 0), stop=(s2 == CHUNK - 1))
            # z (+bias) -> SBUF; partition (hblk, e), free (q, w)
            zsl = zall[:, rg0 * WP:(rg0 + q) * WP]
            nc.scalar.activation(out=zsl,
                                 in_=pt.rearrange("p a b -> p (a b)"),
                                 func=Ident, bias=cvt[:, 0:1])
            nc.vector.bn_stats(out=statsall[:, ti], in_=zsl)

        # Per-partition mean/var over all 2048 elements
        mv = zpool.tile([P, 2], f32)
        nc.vector.bn_aggr(out=mv, in_=statsall)
        # rhs = (mean_p, E[x^2]_p); E2 = var + mean^2 in one fused DVE op
        me2 = zpool.tile([P, 2], f32)
        nc.vector.tensor_copy(me2[:, 0:1], mv[:, 0:1])
        nc.vector.scalar_tensor_tensor(
            out=me2[:, 1:2], in0=mv[:, 0:1], scalar=mv[:, 0:1],
            in1=mv[:, 1:2], op0=mybir.AluOpType.mult,
            op1=mybir.AluOpType.add)
        # Grouped cross-partition average + broadcast in one matmul:
        # gp[p'] = (1/16) * sum_{p in group(p')} me2[p]
        gp = psg.tile([P, 2], f32)
        nc.tensor.matmul(gp, lhsT=gmt, rhs=me2, start=True, stop=True)
        gsb = zpool.tile([P, 2], f32)
        nc.vector.tensor_copy(gsb, gp)
        gmean = gsb[:, 0:1]
        gmsq = zpool.tile([P, 1], f32)
        nc.vector.tensor_mul(gmsq, gmean, gmean)
        gvar = zpool.tile([P, 1], f32)
        nc.vector.tensor_sub(gvar, gsb[:, 1:2], gmsq)
        sd = zpool.tile([P, 1], f32)
        nc.scalar.activation(out=sd, in_=gvar, func=Sqrt, bias=epst)
        rs = zpool.tile([P, 1], f32)
        nc.vector.reciprocal(rs, sd)
        # out = z * A + Bp with A = rsqrt*gamma, Bp = beta - mean*A
        A = zpool.tile([P, 1], f32)
        nc.vector.tensor_mul(A, rs, cvt[:, 1:2])
        mA = zpool.tile([P, 1], f32)
        nc.vector.tensor_mul(mA, gmean, A)
        Bp = zpool.tile([P, 1], f32)
        nc.vector.tensor_sub(Bp, cvt[:, 2:3], mA)

        onorm = zpool.tile([P, RB * WP], f32)
        for ti, q in enumerate(TILE_Q):
            rg0 = rg0s[ti]
            sl = slice(rg0 * WP, (rg0 + q) * WP)
            nc.scalar.activation(out=onorm[:, sl], in_=zall[:, sl],
                                 func=Ident, scale=A, bias=Bp)
            nc.sync.dma_start(out=out[:, sl], in_=onorm[:, sl])

    nc.compile()
    return nc


def _host_weights(fc_wr, fc_wi, fc_br, fc_bi, conv_w, conv_b, gamma, beta):
    fc_wr = np.asarray(fc_wr, np.float64)
    fc_wi = np.asarray(fc_wi, np.float64)
    fc_br = np.asarray(fc_br, np.float64)
    fc_bi = np.asarray(fc_bi, np.float64)
    conv_w = np.asarray(conv_w, np.float64)
    conv_b = np.asarray(conv_b, np.float64)
    gamma = np.asarray(gamma, np.float64)
    beta = np.asarray(beta, np.float64)

    j = np.arange(D_IN)
    F = np.exp(-2j * np.pi * np.outer(j, j) / D_IN) / np.sqrt(D_IN)
    d = np.arange(D)
    G = np.exp(2j * np.pi * np.outer(d, d) / D) / np.sqrt(D)
    Wc = fc_wr + 1j * fc_wi
    bc = fc_br + 1j * fc_bi
    M2 = (np.real(F @ Wc.T @ G) @ conv_w.T).astype(np.float32)  # [256, 16]
    b2 = (np.real(bc @ G) @ conv_w.T + conv_b).astype(np.float32)  # [16]

    # Block-diagonal lhsT: wl[hblk*16+s1, s2, hblk*16+e] = M2[s1*16+s2, e],
    # packed [p, s2*128+m] for a contiguous per-partition SBUF load.
    wl = np.zeros((CHUNK, P, P), np.float32)  # [s2, p, m]
    blk = M2.reshape(CHUNK, CHUNK, D).transpose(1, 0, 2)  # [s2, s1, e]
    for hb in range(8):
        wl[:, hb * 16:hb * 16 + 16, hb * 16:hb * 16 + 16] = blk
    wl = np.ascontiguousarray(wl.transpose(1, 0, 2).reshape(P, CHUNK * P))

    # Group-average + broadcast mask (1/16 where groups match)
    pidx = np.arange(P)
    grp = (pidx % D) // (D // GROUPS)
    gmask = (grp[:, None] == grp[None, :]).astype(np.float32) / 16.0

    e = pidx % D
    consts = np.stack([b2[e], gamma.astype(np.float32)[e],
                       beta.astype(np.float32)[e]], axis=1)  # [128, 3]
    return wl, gmask, consts


def kernel(x, fc_wr, fc_wi, fc_br, fc_bi, conv_w, conv_b, gamma, beta,
           _return_results=False, _trace=False, _mm_dtype="float32r"):
    from concourse.bass_utils import run_bass_kernel_spmd

    if _mm_dtype not in _CACHED_NC:
        _CACHED_NC[_mm_dtype] = _build_nc(_mm_dtype)
    nc = _CACHED_NC[_mm_dtype]

    wl, gmask, consts = _host_weights(fc_wr, fc_wi, fc_br, fc_bi,
                                      conv_w, conv_b, gamma, beta)
    x = np.ascontiguousarray(np.asarray(x, np.float32).reshape(B, H, W))
    in_maps = [{"x": x[b], "wl": wl, "gmask": gmask, "consts": consts}
               for b in range(N_CORES)]
    res = run_bass_kernel_spmd(nc, in_maps, list(range(N_CORES)),
                               trace=_trace)
    # device layout [p=(hblk,e), rg, w] -> [D, HP, WP], hi = rg*8 + hblk
    out = np.stack(
        [res.results[b]["out"].reshape(8, D, RB, WP)
         .transpose(1, 2, 0, 3).reshape(D, HP, WP)
         for b in range(N_CORES)], axis=0)
    if _return_results:
        return out, res
    return out
